# revision 42
# baseline (speedup 1.0000x reference)
"""AdaptiveRankLinear on 8 TRN2 NeuronCores.

y[b,t,o] = sum_i x[b,t,i] * W[o,i] + bias[o],  W = U @ (diag(S) @ Vt)

Sharding: pure data-parallel over batch (B=8 == n_cores); U/S/Vt/bias
replicated. Per core: y_b = (x_b @ Vts^T) @ U^T + bias via the rank-256
bottleneck — 2 chained matmuls instead of materializing the 4096x4096 W.

Key design points:
  - x stored as fp8 e3m4: halves x HBM traffic; mm1 runs mixed-dtype
    (bf16 stationary Vts^T, e3m4 moving x) at full PE rate. One e3m4
    quantization hit ~1.2e-2 rel err vs the 2e-2 gate. The DMA byte/
    descriptor reduction also keeps the chip out of the P0 power
    throttle: 8-core matmul sustains 216ns/512cols (2.4GHz) vs 259ns
    (2.0GHz) with bf16 x.
  - two HWDGE rings: x loads on sync (SP), weights + y stores on scalar
    (ACT) — first matmul's operands arrive in parallel; ut loaded in
    need-ordered column slices.
  - 44 dummy K=1 matmuls warm the PE (HAM ramp ~3.4us at 1.2GHz)
    during the initial DMA wait, so real matmuls start at full clock.
  - PE program: mm1(c0) first, then mm1(c+1) segments interleaved
    BEFORE mm2(c) m-blocks so tt copies always hide under mm2 of the
    previous chunk and DVE evacuation work spreads evenly.
  - psum->sbuf evacuation alternates DVE (fused add) and ScalarE copy +
    DVE bf16 add; bias is host-pre-broadcast and rides the sync ring
    mid-stream; y stores alternate between both HWDGE rings and the
    last two row-tiles store per-1KB-slice so the tail drains in
    parallel.
Compute: f32 PSUM accumulate, bf16 output (host casts back to f32).
rel err ~1.2e-2 vs the 2e-2 gate.
"""

import numpy as np
import ml_dtypes

B, T, IN, OUT, RANK = 8, 2048, 4096, 4096, 256
N_CORES = 8
P = 128
TC = 512               # T chunk (psum bank = 512 f32)
NCHUNK = T // TC       # 4
NIT = IN // P          # 32 contraction tiles for mm1
NRT = RANK // P        # 2 rank tiles
OC = 512               # matmul free-dim max
MT = TC // P           # 4 T-tiles per chunk
NG = 4                 # x/vtst load groups per chunk
GN = NIT // NG         # IN tiles per load group (8)
SEG = NIT // MT        # mm1 rows per interleave segment (8)
NWARM = 12             # PE warmup matmuls (4 ramp-width + 8 full-width)

BF16 = ml_dtypes.bfloat16
E3M4 = ml_dtypes.float8_e3m4

_CACHE = {}


def _build():
    import concourse.bacc as bacc
    import concourse.bass as bass
    import concourse.tile as tile
    from concourse import mybir

    f32 = mybir.dt.float32
    bf16 = mybir.dt.bfloat16
    f8e3 = mybir.dt.float8e3

    from concourse import library_config

    nc = bacc.Bacc("TRN2", target_bir_lowering=False, debug=False,
                   num_devices=N_CORES)
    # packed layouts (host-prepped): per (chunk, group) x block is
    # [P, GN*TC] e3m4; per group vtst block is [P, GN*RANK] bf16 —
    # contiguous per-partition rows = descriptor-friendly DMA.
    xp = nc.dram_tensor("xp", [NCHUNK * NG * P, GN * TC], f8e3,
                        kind="ExternalInput")
    vp = nc.dram_tensor("vp", [NG * P, GN * RANK], f8e3,
                        kind="ExternalInput")
    ut = nc.dram_tensor("ut", [RANK, OUT], bf16, kind="ExternalInput")
    ut8 = nc.dram_tensor("ut8", [RANK, OUT], f8e3, kind="ExternalInput")
    bias1 = nc.dram_tensor("bias1", [1, OUT], bf16, kind="ExternalInput")
    out = nc.dram_tensor("out", [T, OUT], bf16, kind="ExternalOutput")

    with tile.TileContext(nc) as tc:
        with (
            tc.tile_pool(name="weights", bufs=1) as wpool,
            tc.tile_pool(name="xin", bufs=12) as xpool,
            tc.tile_pool(name="tt", bufs=3) as tpool,
            tc.tile_pool(name="yout", bufs=4) as ypool,
            tc.tile_pool(name="pt", bufs=1, space=bass.MemorySpace.PSUM) as ptp,
            tc.tile_pool(name="py", bufs=3, space=bass.MemorySpace.PSUM) as pyp,
        ):
            # Pool ucode: load the one library covering BOTH
            # partition_broadcast and tensor_tensor up front — the lazy
            # per-instruction load otherwise lands mid-kernel (observed:
            # ~5.7us ucode DMA at ~118us stalling the psum-recycle chain
            # for the last chunk's evacs, PE idle 5.5us + util re-ramp).
            nc.gpsimd.load_library(library_config.proxy)

            # warm tiles: K=128 x 512-col dummies draw real PE power so the
            # HAM clock ramp completes during the initial DMA wait (K=1
            # dummies only toggle 128 of 16384 cells -> ramp lagged ~3us
            # into real work). Distinct mantissa-rich values maximize bit
            # toggling in the multipliers.
            warm_w = wpool.tile([P, P], bf16, tag="warm_w")
            warm_m = wpool.tile([P, 512], bf16, tag="warm_m")
            nc.vector.memset(warm_w[:], 1.3779297)
            nc.vector.memset(warm_m[:], -0.7392578)

            # ---- loads: both rings carry ~1.5MB of chunk-0-critical data
            # (sync: xg groups 0-2; scalar: fp8 vtst + xg group 3) so c0's
            # operands finish together ~17us instead of 20.5 with x all on
            # one ring. FIFO need-order on each ring.
            def load_x_group(c, g, parts=1, eng=None):
                eng = eng or nc.sync
                xg = xpool.tile([P, GN * TC], f8e3, tag="xg",
                                name=f"xg_{c}_{g}")
                r0 = (c * NG + g) * P
                w = GN * TC // parts
                for hh in range(parts):
                    eng.dma_start(xg[:, hh * w:(hh + 1) * w],
                                  xp[r0:r0 + P, hh * w:(hh + 1) * w])
                return xg

            # bias loads as a single 8KB row FIRST on the sync ring and
            # broadcasts on-chip via the idle Pool engine (vs. 1MB
            # host-pre-broadcast riding the ring mid-stream). First evac
            # needs bias_sb at ~27us; this has it ready by ~13us.
            bias1_sb = wpool.tile([1, OUT], bf16, tag="bias1")
            bias_sb = wpool.tile([P, OUT], bf16, tag="bias")
            nc.sync.dma_start(bias1_sb[:], bias1[:, :])
            nc.gpsimd.partition_broadcast(bias_sb[:], bias1_sb[:])

            vtst_g = []
            for g in range(NG):
                vw = wpool.tile([P, GN * RANK], f8e3, tag=f"vtst{g}",
                                name=f"vtst{g}")
                parts = 4 if g == 0 else (2 if g == 1 else 1)
                wv = GN * RANK // parts
                for hh in range(parts):
                    nc.scalar.dma_start(vw[:, hh * wv:(hh + 1) * wv],
                                        vp[g * P:(g + 1) * P,
                                           hh * wv:(hh + 1) * wv])
                vtst_g.append(vw)

            xc = {}
            xc[(0, 0)] = load_x_group(0, 0, parts=4)
            xc[(0, 1)] = load_x_group(0, 1, parts=2)
            xc[(0, 2)] = load_x_group(0, 2, parts=2)
            xc[(0, 3)] = load_x_group(0, 3, parts=2, eng=nc.scalar)

            # chunk-0's mm2 uses an e3m4 copy of ut (1MB instead of 2MB in
            # the HBM-saturated early window; costs one extra fp8 weight
            # hit on 25% of output rows vs the 2e-2 gate). Split across
            # BOTH rings in mm2's oh need-order so each ring carries ~2MB
            # of c0-critical bytes. The bf16 ut loads after, needed first
            # by chunk 1's mm2.
            ut8_sb = [wpool.tile([P, OUT], f8e3, tag=f"ut8{j}",
                                 name=f"ut8{j}") for j in range(NRT)]
            for o0, o1 in ((0, 2048), (2048, OUT)):
                for j in range(NRT):
                    nc.scalar.dma_start(ut8_sb[j][:, o0:o1],
                                        ut8[j * P:(j + 1) * P, o0:o1])
            ut_sb = [wpool.tile([P, OUT], bf16, tag=f"ut{j}", name=f"ut{j}")
                     for j in range(NRT)]
            for o0, o1 in ((0, 2048), (2048, OUT)):
                for j in range(NRT):
                    nc.scalar.dma_start(ut_sb[j][:, o0:o1],
                                        ut[j * P:(j + 1) * P, o0:o1])

            xc[(1, 0)] = load_x_group(1, 0)
            xc[(1, 1)] = load_x_group(1, 1)
            xc[(1, 2)] = load_x_group(1, 2)
            xc[(1, 3)] = load_x_group(1, 3)
            for c in range(2, NCHUNK):
                for g in range(NG):
                    xc[(c, g)] = load_x_group(c, g)

            tts = {}

            # ---- dummy matmuls warm the PE HAM during the DMA wait:
            # first operands land ~10us; dummies run from ~7.1us. Widths
            # step up 128->512 so the power draw ramps instead of slamming
            # from idle to full — an abrupt slam sometimes trips the chip
            # power manager into a k=4/8 utilization limit for ~7-10us
            # right as real work starts.
            pd = pyp.tile([P, 1024], f32, tag="py", name="warm")
            for w in (128, 128, 256, 256) + (512,) * (NWARM - 4):
                nc.tensor.matmul(pd[:, :w], warm_w[:, :], warm_m[:, :w],
                                 start=True, stop=True)

            def emit_mm1(c, n0, n1):
                if n0 == 0:
                    tts[c] = {"pt": [
                        ptp.tile([P, TC], f32, tag=f"pt{j}", name=f"pt{j}_{c}")
                        for j in range(NRT)]}
                pt = tts[c]["pt"]
                for n in range(n0, n1):
                    g, nl = divmod(n, GN)
                    for j in range(NRT):
                        nc.tensor.matmul(
                            pt[j][:],
                            vtst_g[g][:, nl * RANK + j * P:
                                      nl * RANK + (j + 1) * P],
                            xc[(c, g)][:, nl * TC:(nl + 1) * TC],
                            start=(n == 0), stop=(n == NIT - 1))
                if n1 == NIT:
                    tts[c]["tt"] = []
                    for j in range(NRT):
                        ttj = tpool.tile([P, TC], bf16, tag=f"tt{j}",
                                         name=f"tt{j}_{c}")
                        nc.vector.tensor_copy(ttj[:], pt[j][:])
                        tts[c]["tt"].append(ttj)

            def emit_mm2_block(c, m):
                # Evac engine budget: in c0-c2 each mm2 block shares its
                # window with an mm1 segment (6.9us/block) so the baseline
                # alternate DVE-f32add / ACT-copy+DVE-bf16add keeps up. In
                # c3 (no mm1) blocks take 3.46us and that scheme runs DVE
                # at 109% -> evacs lagged ~2.5us and 1.9MB of y drained
                # after the last matmul. c3 instead splits 4 ways: two
                # DVE f32-adds, one ACT copy + Pool bf16-add, one ACT copy
                # + DVE bf16-add (DVE 90%, ACT 65%, Pool 67%). Store
                # triggers stay off the ACT queue's copy path where they
                # would head-of-line block behind slower adders.
                tt = tts[c]["tt"]
                u_sb = ut8_sb if c == 0 else ut_sb
                last_c = c == NCHUNK - 1
                row = (c * MT + m) * P
                y = ypool.tile([P, OUT], bf16, tag="y")
                for oh in range(OUT // 1024):
                    py = pyp.tile([P, 1024], f32, tag="py")
                    for oo in range(2):
                        for j in range(NRT):
                            o0 = oh * 1024 + oo * OC
                            nc.tensor.matmul(
                                py[:, oo * OC:(oo + 1) * OC],
                                tt[j][:, m * P:(m + 1) * P],
                                u_sb[j][:, o0:o0 + OC],
                                start=(j == 0), stop=(j == NRT - 1))
                    ys = y[:, oh * 1024:(oh + 1) * 1024]
                    bs = bias_sb[:, oh * 1024:(oh + 1) * 1024]
                    if not last_c:
                        if (m * 4 + oh) % 2 == 0:
                            nc.scalar.copy(ys, py[:])
                            nc.vector.tensor_add(ys, ys, bs)
                        else:
                            nc.vector.tensor_add(ys, py[:], bs)
                    else:
                        # c3 evac 4-way split (DVE 90%, ACT 82%, Pool 67%)
                        # with per-oh stores so each 256KB piece flows as
                        # soon as its evac lands. Stores use only the SP +
                        # ACT hardware DGE queues (a gpsimd dma_start lands
                        # on the SOFTWARE queue: one engine, ~25GB/s). In
                        # the very last block oh2/oh3 go straight to DVE
                        # f32-adds so the final chain after the last matmul
                        # is two back-to-back DVE ops, not an ACT+DVE relay.
                        last_m = m == MT - 1
                        eng = None
                        if oh == 0:
                            nc.scalar.copy(ys, py[:])
                            nc.gpsimd.tensor_add(ys, ys, bs)
                            eng = nc.sync
                        elif oh == 1:
                            nc.vector.tensor_add(ys, py[:], bs)
                            eng = nc.scalar
                        elif oh == 2:
                            nc.scalar.copy(ys, py[:])
                            if last_m:
                                # defer oh2's bias add until after oh3's
                                # f32-add is queued on DVE: the final DVE
                                # chain is then f32(oh3) -> bf16(oh2)
                                # instead of three serialized f32-adds.
                                defer = (ys, bs)
                            else:
                                nc.vector.tensor_add(ys, ys, bs)
                                eng = nc.sync
                        else:
                            nc.vector.tensor_add(ys, py[:], bs)
                            eng = nc.sync
                        if eng is not None:
                            eng.dma_start(
                                out[row:row + P, oh * 1024:(oh + 1) * 1024],
                                y[:, oh * 1024:(oh + 1) * 1024])
                        if last_m and oh == 3:
                            ys2, bs2 = defer
                            nc.vector.tensor_add(ys2, ys2, bs2)
                            nc.scalar.dma_start(
                                out[row:row + P, 2 * 1024:3 * 1024],
                                y[:, 2 * 1024:3 * 1024])
                if not last_c:
                    # full-row stores: one cheap trigger per MB; alternate
                    # rings (sync is idle after the x loads drain)
                    eng = nc.scalar if (c * MT + m) % 2 == 0 else nc.sync
                    eng.dma_start(out[row:row + P, :], y[:])

            # ---- PE program ----
            emit_mm1(0, 0, NIT)
            for c in range(NCHUNK):
                for m in range(MT):
                    if c + 1 < NCHUNK:
                        emit_mm1(c + 1, m * SEG, (m + 1) * SEG)
                    emit_mm2_block(c, m)

    nc.compile()
    return nc


def _prep_in_maps(x, U, S, Vt, bias):
    x = np.asarray(x, dtype=np.float32)
    U = np.asarray(U, dtype=np.float32)
    S = np.asarray(S, dtype=np.float32)
    Vt = np.asarray(Vt, dtype=np.float32)
    bias = np.asarray(bias, dtype=np.float32)

    # vtst ships as e3m4 (halves the c0-critical weight bytes). Per-rank
    # pow2 normalization keeps every column in fp8's normal range; the
    # inverse scale folds exactly into U's rank rows (pow2 = lossless in
    # bf16, and bounded to 2^5 so the e3m4 ut8 copy stays out of deep
    # subnormals).
    vtstT = np.ascontiguousarray((S[:, None] * Vt).T)              # [IN,R] f32
    colmax = np.abs(vtstT).max(axis=0)                             # [R]
    k = np.clip(np.floor(np.log2(8.0 / np.maximum(colmax, 1e-30))),
                -2, 5)                                             # [R]
    scale = np.exp2(k).astype(np.float32)
    vtstT = (vtstT * scale[None, :]).astype(E3M4)
    v4 = np.asarray(vtstT).reshape(NIT, P, RANK)
    vp_np = np.concatenate(
        [v4[g * GN:(g + 1) * GN].transpose(1, 0, 2).reshape(P, GN * RANK)
         for g in range(NG)], axis=0)                              # [NG*P, GN*R]
    utT = np.ascontiguousarray(U.T) * (1.0 / scale)[:, None]       # [R,OUT]
    ut_np = utT.astype(BF16)
    ut8_np = utT.astype(E3M4)
    bias1_np = np.ascontiguousarray(bias[None, :].astype(BF16))    # [1,OUT]

    in_maps = []
    for c in range(N_CORES):
        xT = np.ascontiguousarray(x[c].T).astype(E3M4)             # [IN,T]
        x4 = xT.reshape(NIT, P, T)
        blocks = []
        for cc in range(NCHUNK):
            for g in range(NG):
                blocks.append(
                    x4[g * GN:(g + 1) * GN, :, cc * TC:(cc + 1) * TC]
                    .transpose(1, 0, 2).reshape(P, GN * TC))
        xp_np = np.concatenate(blocks, axis=0)        # [NCHUNK*NG*P, GN*TC]
        in_maps.append({"xp": xp_np, "vp": vp_np, "ut": ut_np,
                        "ut8": ut8_np, "bias1": bias1_np})
    return in_maps


def _run(inputs, trace=False, trace_kwargs=None):
    import concourse.bass_utils as bass_utils
    if trace:
        bass_utils.upload_artifacts = lambda tmpdir: tmpdir
    if "nc" not in _CACHE:
        _CACHE["nc"] = _build()
    nc = _CACHE["nc"]
    in_maps = _prep_in_maps(**inputs)
    res = bass_utils.run_bass_kernel_spmd(
        nc, in_maps, core_ids=list(range(N_CORES)), trace=trace,
        **(trace_kwargs or {}))
    y = np.stack([res.results[c]["out"] for c in range(N_CORES)],
                 axis=0).astype(np.float32)
    return y, res


def kernel(**inputs) -> np.ndarray:
    y, _ = _run(inputs, trace=False)
    return y



# revision 43
# speedup vs baseline: 1.0454x; 1.0454x over previous
"""AdaptiveRankLinear on 8 TRN2 NeuronCores.

y[b,t,o] = sum_i x[b,t,i] * W[o,i] + bias[o],  W = U @ (diag(S) @ Vt)

Sharding: pure data-parallel over batch (B=8 == n_cores); U/S/Vt/bias
replicated. Per core: y_b = (x_b @ Vts^T) @ U^T + bias via the rank-256
bottleneck — 2 chained matmuls instead of materializing the 4096x4096 W.

Key design points:
  - x stored as fp8 e3m4: halves x HBM traffic; mm1 runs mixed-dtype
    (bf16 stationary Vts^T, e3m4 moving x) at full PE rate. One e3m4
    quantization hit ~1.2e-2 rel err vs the 2e-2 gate. The DMA byte/
    descriptor reduction also keeps the chip out of the P0 power
    throttle: 8-core matmul sustains 216ns/512cols (2.4GHz) vs 259ns
    (2.0GHz) with bf16 x.
  - two HWDGE rings: x loads on sync (SP), weights + y stores on scalar
    (ACT) — first matmul's operands arrive in parallel; ut loaded in
    need-ordered column slices.
  - 44 dummy K=1 matmuls warm the PE (HAM ramp ~3.4us at 1.2GHz)
    during the initial DMA wait, so real matmuls start at full clock.
  - PE program: mm1(c0) first, then mm1(c+1) segments interleaved
    BEFORE mm2(c) m-blocks so tt copies always hide under mm2 of the
    previous chunk and DVE evacuation work spreads evenly.
  - psum->sbuf evacuation alternates DVE (fused add) and ScalarE copy +
    DVE bf16 add; bias is host-pre-broadcast and rides the sync ring
    mid-stream; y stores alternate between both HWDGE rings and the
    last two row-tiles store per-1KB-slice so the tail drains in
    parallel.
Compute: f32 PSUM accumulate, bf16 output (host casts back to f32).
rel err ~1.2e-2 vs the 2e-2 gate.
"""

import numpy as np
import ml_dtypes

B, T, IN, OUT, RANK = 8, 2048, 4096, 4096, 256
N_CORES = 8
P = 128
TC = 512               # T chunk (psum bank = 512 f32)
NCHUNK = T // TC       # 4
NIT = IN // P          # 32 contraction tiles for mm1
NRT = RANK // P        # 2 rank tiles
OC = 512               # matmul free-dim max
MT = TC // P           # 4 T-tiles per chunk
NG = 4                 # x/vtst load groups per chunk
GN = NIT // NG         # IN tiles per load group (8)
SEG = NIT // MT        # mm1 rows per interleave segment (8)
NWARM = 8              # full-width PE warmup matmuls

BF16 = ml_dtypes.bfloat16
E3M4 = ml_dtypes.float8_e3m4

_CACHE = {}


def _build():
    import concourse.bacc as bacc
    import concourse.bass as bass
    import concourse.tile as tile
    from concourse import mybir

    f32 = mybir.dt.float32
    bf16 = mybir.dt.bfloat16
    f8e3 = mybir.dt.float8e3

    from concourse import library_config

    nc = bacc.Bacc("TRN2", target_bir_lowering=False, debug=False,
                   num_devices=N_CORES)
    # packed layouts (host-prepped): per (chunk, group) x block is
    # [P, GN*TC] e3m4; per group vtst block is [P, GN*RANK] bf16 —
    # contiguous per-partition rows = descriptor-friendly DMA.
    xp = nc.dram_tensor("xp", [NCHUNK * NG * P, GN * TC], f8e3,
                        kind="ExternalInput")
    vp = nc.dram_tensor("vp", [NG * P, GN * RANK], f8e3,
                        kind="ExternalInput")
    ut = nc.dram_tensor("ut", [RANK, OUT], bf16, kind="ExternalInput")
    ut8 = nc.dram_tensor("ut8", [RANK, OUT], f8e3, kind="ExternalInput")
    bias1 = nc.dram_tensor("bias1", [1, OUT], bf16, kind="ExternalInput")
    out = nc.dram_tensor("out", [T, OUT], bf16, kind="ExternalOutput")

    with tile.TileContext(nc) as tc:
        with (
            tc.tile_pool(name="weights", bufs=1) as wpool,
            tc.tile_pool(name="xin", bufs=12) as xpool,
            tc.tile_pool(name="tt", bufs=3) as tpool,
            tc.tile_pool(name="yout", bufs=4) as ypool,
            tc.tile_pool(name="pt", bufs=1, space=bass.MemorySpace.PSUM) as ptp,
            tc.tile_pool(name="py", bufs=3, space=bass.MemorySpace.PSUM) as pyp,
        ):
            # Pool ucode: load the one library covering BOTH
            # partition_broadcast and tensor_tensor up front — the lazy
            # per-instruction load otherwise lands mid-kernel (observed:
            # ~5.7us ucode DMA at ~118us stalling the psum-recycle chain
            # for the last chunk's evacs, PE idle 5.5us + util re-ramp).
            nc.gpsimd.load_library(library_config.proxy)

            # warm tiles: K=128 x 512-col dummies draw real PE power so the
            # HAM clock ramp completes during the initial DMA wait (K=1
            # dummies only toggle 128 of 16384 cells -> ramp lagged ~3us
            # into real work). Distinct mantissa-rich values maximize bit
            # toggling in the multipliers.
            warm_w = wpool.tile([P, P], bf16, tag="warm_w")
            warm_m = wpool.tile([P, 512], bf16, tag="warm_m")
            nc.vector.memset(warm_w[:], 1.3779297)
            nc.vector.memset(warm_m[:], -0.7392578)

            # ---- loads: both rings carry ~1.5MB of chunk-0-critical data
            # (sync: xg groups 0-2; scalar: fp8 vtst + xg group 3) so c0's
            # operands finish together ~17us instead of 20.5 with x all on
            # one ring. FIFO need-order on each ring.
            def load_x_group(c, g, parts=1, eng=None):
                eng = eng or nc.sync
                xg = xpool.tile([P, GN * TC], f8e3, tag="xg",
                                name=f"xg_{c}_{g}")
                r0 = (c * NG + g) * P
                w = GN * TC // parts
                for hh in range(parts):
                    eng.dma_start(xg[:, hh * w:(hh + 1) * w],
                                  xp[r0:r0 + P, hh * w:(hh + 1) * w])
                return xg

            vtst_g = []
            for g in range(NG):
                vw = wpool.tile([P, GN * RANK], f8e3, tag=f"vtst{g}",
                                name=f"vtst{g}")
                parts = 4 if g == 0 else (2 if g == 1 else 1)
                wv = GN * RANK // parts
                for hh in range(parts):
                    nc.scalar.dma_start(vw[:, hh * wv:(hh + 1) * wv],
                                        vp[g * P:(g + 1) * P,
                                           hh * wv:(hh + 1) * wv])
                vtst_g.append(vw)

            xc = {}
            xc[(0, 0)] = load_x_group(0, 0, parts=4)
            xc[(0, 1)] = load_x_group(0, 1, parts=2)
            xc[(0, 2)] = load_x_group(0, 2, parts=2)
            xc[(0, 3)] = load_x_group(0, 3, parts=2, eng=nc.scalar)

            # chunk-0's mm2 uses an e3m4 copy of ut (1MB instead of 2MB in
            # the HBM-saturated early window; costs one extra fp8 weight
            # hit on 25% of output rows vs the 2e-2 gate). Split across
            # BOTH rings in mm2's oh need-order so each ring carries ~2MB
            # of c0-critical bytes. The bf16 ut loads after, needed first
            # by chunk 1's mm2.
            ut8_sb = [wpool.tile([P, OUT], f8e3, tag=f"ut8{j}",
                                 name=f"ut8{j}") for j in range(NRT)]
            for o0, o1 in ((0, 2048), (2048, OUT)):
                for j in range(NRT):
                    nc.scalar.dma_start(ut8_sb[j][:, o0:o1],
                                        ut8[j * P:(j + 1) * P, o0:o1])
            ut_sb = [wpool.tile([P, OUT], bf16, tag=f"ut{j}", name=f"ut{j}")
                     for j in range(NRT)]
            for o0, o1 in ((0, 2048), (2048, OUT)):
                for j in range(NRT):
                    nc.scalar.dma_start(ut_sb[j][:, o0:o1],
                                        ut[j * P:(j + 1) * P, o0:o1])

            bias1_sb = wpool.tile([1, OUT], bf16, tag="bias1")
            bias_sb = wpool.tile([P, OUT], bf16, tag="bias")
            nc.sync.dma_start(bias1_sb[:], bias1[:, :])
            nc.gpsimd.partition_broadcast(bias_sb[:], bias1_sb[:])
            xc[(1, 0)] = load_x_group(1, 0)
            xc[(1, 1)] = load_x_group(1, 1)
            xc[(1, 2)] = load_x_group(1, 2)
            xc[(1, 3)] = load_x_group(1, 3)
            for c in range(2, NCHUNK):
                for g in range(NG):
                    xc[(c, g)] = load_x_group(c, g)

            tts = {}

            # ---- dummy matmuls warm the PE HAM during the DMA wait:
            # first operands land ~10us; dummies run from ~7.1us. Widths
            # step up 128->512 so the power draw ramps instead of slamming
            # from idle to full — an abrupt slam sometimes trips the chip
            # power manager into a k=4/8 utilization limit for ~7-10us
            # right as real work starts.
            pd = pyp.tile([P, 1024], f32, tag="py", name="warm")
            for _ in range(NWARM):
                nc.tensor.matmul(pd[:, :OC], warm_w[:, :], warm_m[:, :OC],
                                 start=True, stop=True)

            def emit_mm1(c, n0, n1):
                if n0 == 0:
                    tts[c] = {"pt": [
                        ptp.tile([P, TC], f32, tag=f"pt{j}", name=f"pt{j}_{c}")
                        for j in range(NRT)]}
                pt = tts[c]["pt"]
                for n in range(n0, n1):
                    g, nl = divmod(n, GN)
                    for j in range(NRT):
                        nc.tensor.matmul(
                            pt[j][:],
                            vtst_g[g][:, nl * RANK + j * P:
                                      nl * RANK + (j + 1) * P],
                            xc[(c, g)][:, nl * TC:(nl + 1) * TC],
                            start=(n == 0), stop=(n == NIT - 1))
                if n1 == NIT:
                    tts[c]["tt"] = []
                    for j in range(NRT):
                        ttj = tpool.tile([P, TC], bf16, tag=f"tt{j}",
                                         name=f"tt{j}_{c}")
                        nc.vector.tensor_copy(ttj[:], pt[j][:])
                        tts[c]["tt"].append(ttj)

            def emit_mm2_block(c, m):
                # Evac engine budget: in c0-c2 each mm2 block shares its
                # window with an mm1 segment (6.9us/block) so the baseline
                # alternate DVE-f32add / ACT-copy+DVE-bf16add keeps up. In
                # c3 (no mm1) blocks take 3.46us and that scheme runs DVE
                # at 109% -> evacs lagged ~2.5us and 1.9MB of y drained
                # after the last matmul. c3 instead splits 4 ways: two
                # DVE f32-adds, one ACT copy + Pool bf16-add, one ACT copy
                # + DVE bf16-add (DVE 90%, ACT 65%, Pool 67%). Store
                # triggers stay off the ACT queue's copy path where they
                # would head-of-line block behind slower adders.
                tt = tts[c]["tt"]
                u_sb = ut8_sb if c == 0 else ut_sb
                last_c = c == NCHUNK - 1
                row = (c * MT + m) * P
                y = ypool.tile([P, OUT], bf16, tag="y")
                for oh in range(OUT // 1024):
                    py = pyp.tile([P, 1024], f32, tag="py")
                    for oo in range(2):
                        for j in range(NRT):
                            o0 = oh * 1024 + oo * OC
                            nc.tensor.matmul(
                                py[:, oo * OC:(oo + 1) * OC],
                                tt[j][:, m * P:(m + 1) * P],
                                u_sb[j][:, o0:o0 + OC],
                                start=(j == 0), stop=(j == NRT - 1))
                    ys = y[:, oh * 1024:(oh + 1) * 1024]
                    bs = bias_sb[:, oh * 1024:(oh + 1) * 1024]
                    if not last_c:
                        if (m * 4 + oh) % 2 == 0:
                            nc.scalar.copy(ys, py[:])
                            nc.vector.tensor_add(ys, ys, bs)
                        else:
                            nc.vector.tensor_add(ys, py[:], bs)
                    else:
                        # c3 evac 4-way split (DVE 90%, ACT 82%, Pool 67%)
                        # with per-oh stores so each 256KB piece flows as
                        # soon as its evac lands. Stores use only the SP +
                        # ACT hardware DGE queues (a gpsimd dma_start lands
                        # on the SOFTWARE queue: one engine, ~25GB/s). In
                        # the very last block oh2/oh3 go straight to DVE
                        # f32-adds so the final chain after the last matmul
                        # is two back-to-back DVE ops, not an ACT+DVE relay.
                        last_m = m == MT - 1
                        if oh == 0:
                            nc.scalar.copy(ys, py[:])
                            nc.gpsimd.tensor_add(ys, ys, bs)
                            eng = nc.sync
                        elif oh == 1:
                            nc.vector.tensor_add(ys, py[:], bs)
                            eng = nc.scalar
                        elif oh == 2:
                            if last_m:
                                nc.vector.tensor_add(ys, py[:], bs)
                                eng = nc.scalar
                            else:
                                nc.scalar.copy(ys, py[:])
                                nc.vector.tensor_add(ys, ys, bs)
                                eng = nc.sync
                        else:
                            nc.vector.tensor_add(ys, py[:], bs)
                            eng = nc.sync
                        eng.dma_start(
                            out[row:row + P, oh * 1024:(oh + 1) * 1024],
                            y[:, oh * 1024:(oh + 1) * 1024])
                if not last_c:
                    # full-row stores: one cheap trigger per MB; alternate
                    # rings (sync is idle after the x loads drain)
                    eng = nc.scalar if (c * MT + m) % 2 == 0 else nc.sync
                    eng.dma_start(out[row:row + P, :], y[:])

            # ---- PE program ----
            emit_mm1(0, 0, NIT)
            for c in range(NCHUNK):
                for m in range(MT):
                    if c + 1 < NCHUNK:
                        emit_mm1(c + 1, m * SEG, (m + 1) * SEG)
                    emit_mm2_block(c, m)

    nc.compile()
    return nc


def _prep_in_maps(x, U, S, Vt, bias):
    x = np.asarray(x, dtype=np.float32)
    U = np.asarray(U, dtype=np.float32)
    S = np.asarray(S, dtype=np.float32)
    Vt = np.asarray(Vt, dtype=np.float32)
    bias = np.asarray(bias, dtype=np.float32)

    # vtst ships as e3m4 (halves the c0-critical weight bytes). Per-rank
    # pow2 normalization keeps every column in fp8's normal range; the
    # inverse scale folds exactly into U's rank rows (pow2 = lossless in
    # bf16, and bounded to 2^5 so the e3m4 ut8 copy stays out of deep
    # subnormals).
    vtstT = np.ascontiguousarray((S[:, None] * Vt).T)              # [IN,R] f32
    colmax = np.abs(vtstT).max(axis=0)                             # [R]
    k = np.clip(np.floor(np.log2(8.0 / np.maximum(colmax, 1e-30))),
                -2, 5)                                             # [R]
    scale = np.exp2(k).astype(np.float32)
    vtstT = (vtstT * scale[None, :]).astype(E3M4)
    v4 = np.asarray(vtstT).reshape(NIT, P, RANK)
    vp_np = np.concatenate(
        [v4[g * GN:(g + 1) * GN].transpose(1, 0, 2).reshape(P, GN * RANK)
         for g in range(NG)], axis=0)                              # [NG*P, GN*R]
    utT = np.ascontiguousarray(U.T) * (1.0 / scale)[:, None]       # [R,OUT]
    ut_np = utT.astype(BF16)
    ut8_np = utT.astype(E3M4)
    bias1_np = np.ascontiguousarray(bias[None, :].astype(BF16))    # [1,OUT]

    in_maps = []
    for c in range(N_CORES):
        xT = np.ascontiguousarray(x[c].T).astype(E3M4)             # [IN,T]
        x4 = xT.reshape(NIT, P, T)
        blocks = []
        for cc in range(NCHUNK):
            for g in range(NG):
                blocks.append(
                    x4[g * GN:(g + 1) * GN, :, cc * TC:(cc + 1) * TC]
                    .transpose(1, 0, 2).reshape(P, GN * TC))
        xp_np = np.concatenate(blocks, axis=0)        # [NCHUNK*NG*P, GN*TC]
        in_maps.append({"xp": xp_np, "vp": vp_np, "ut": ut_np,
                        "ut8": ut8_np, "bias1": bias1_np})
    return in_maps


def _run(inputs, trace=False, trace_kwargs=None):
    import concourse.bass_utils as bass_utils
    if trace:
        bass_utils.upload_artifacts = lambda tmpdir: tmpdir
    if "nc" not in _CACHE:
        _CACHE["nc"] = _build()
    nc = _CACHE["nc"]
    in_maps = _prep_in_maps(**inputs)
    res = bass_utils.run_bass_kernel_spmd(
        nc, in_maps, core_ids=list(range(N_CORES)), trace=trace,
        **(trace_kwargs or {}))
    y = np.stack([res.results[c]["out"] for c in range(N_CORES)],
                 axis=0).astype(np.float32)
    return y, res


def kernel(**inputs) -> np.ndarray:
    y, _ = _run(inputs, trace=False)
    return y

